# revision 1
# baseline (speedup 1.0000x reference)
"""Chamfer loss kernel for Trainium2 (8 NeuronCores, SPMD).

Problem: loss = cd(coarse, gt) + alpha * cd(fine, gt) where
  cd(x, gt) = mean(sqrt(min_x |gt - x|^2)) + 0.1 * mean(sqrt(min_gt |x - gt|^2))

Sharding: core i -> (batch b = i//2, half h = i%2). Each core processes its
half of the query rows (fine: 4096, coarse: 512) against the FULL gt set
(8192) of its batch, block-wise.

Distance matrix D[q, g] = |q|^2 + |g|^2 - 2 q.g via a K=16 fp16
split-precision matmul: each fp32 value v is split as v = vh + vl (two fp16
halves, 22 mantissa bits total); all four cross products (hh, hl, lh, ll)
are separate contraction rows, so products are exact in the fp32 PSUM
accumulation and D is fp32-grade while the PE streams at full 16-bit rate
(fp32 matmul is ~4x slower).

  k 0-2 : W=-2qh   S=gh      k 9-11: W=-2ql   S=gl
  k 3-5 : W=-2qh   S=gl      k 12  : W=nq_h   S=1
  k 6-8 : W=-2ql   S=gh      k 13  : W=nq_l   S=1
                             k 14  : W=1      S=ng_h
                             k 15  : W=1      S=ng_l

Row-mins (per query, over gt): DVE tensor_tensor_reduce on each PSUM group
(also writes an fp16 copy of D to SBUF). Col-mins (per gt, over queries):
running elementwise min over that copy, finalized by PE-transpose + DVE
reduce. Host combines the two halves per batch, clamps, sqrts, means.

Point order is permuted on-chip (contiguous DMA + PE transpose instead of
8192-descriptor strided DMAs); min is order-invariant so the host just
reshapes accordingly.
"""

import os
import sys

import numpy as np

for _p in ("/opt/trn_rl_repo",):
    if _p not in sys.path:
        sys.path.insert(0, _p)

import concourse.bacc as bacc
import concourse.tile as tile
from concourse import masks, mybir
from concourse.bass_utils import run_bass_kernel_spmd

F32 = mybir.dt.float32
F16 = mybir.dt.float16
BIG = 1.0e30


def _install_ntff_hook():
    """The agent image's antenv lacks axon_hooks, which disables NTFF
    profiling under axon. Recreate the module and wire the ctypes hook
    from the boot package so trace=True yields exec_time_ns."""
    try:
        from antenv.axon_hooks import get_axon_ntff_profile_hook  # noqa: F401
        return
    except ImportError:
        pass
    import types

    import antenv

    mod = types.ModuleType("antenv.axon_hooks")
    _holder = {}
    mod.set_axon_ntff_profile_hook = lambda h: _holder.__setitem__("h", h)
    mod.get_axon_ntff_profile_hook = lambda: _holder.get("h")
    sys.modules["antenv.axon_hooks"] = mod
    antenv.axon_hooks = mod
    try:
        if "/root/.axon_site" not in sys.path:
            sys.path.insert(0, "/root/.axon_site")
        from trn_agent_boot.trn_boot import _ntff_profile_via_ctypes
        hook = _ntff_profile_via_ctypes("/opt/axon/libaxon_pjrt.so")
        if hook is not None:
            mod.set_axon_ntff_profile_hook(hook)
    except Exception as e:  # profiling is best-effort; run still works
        print(f"ntff hook install failed: {e}", file=sys.stderr)


_install_ntff_hook()

# Problem constants (hardcoded per contract)
B = 4
NC_PTS = 1024  # coarse points per batch
NF_PTS = 8192  # fine points per batch
NG_PTS = 8192  # gt points per batch
NCORES = 8

NF_H = NF_PTS // 2  # 4096
NC_H = NC_PTS // 2  # 512

K = 16              # contraction rows of the split-precision matmul
GRP = 2048          # free-dim columns per DVE op (4 PSUM banks)
NGRP = NG_PTS // GRP
FCH = NF_H // 128   # 32 fine chunks
CCH = NC_H // 128   # 4 coarse chunks
TBLK = NG_PTS // 128  # 64 transpose blocks for col-min extraction

# "fast": ACT copies each PSUM group to an fp16 scratch; DVE does the exact
#   fp32 row-min reduce from PSUM plus a 2x-rate fp16 col-min update.
# "exact": all-fp32 DVE path (tensor_tensor + reduce straight from PSUM).
MODE = os.environ.get("CHAMFER_MODE", "fast")

OUT_COLS = FCH + CCH + TBLK + TBLK

LAST_EXEC_NS = None
LAST_RESULTS = None

_CACHE = {}

# (source_idx, is_hi) -> destination rows, for query (W) and gt (S) tiles.
# source_idx: 0..2 = x/y/z coordinate, 3 = squared norm.
_W_ROWS = {
    (0, True): (0, 3), (1, True): (1, 4), (2, True): (2, 5),
    (0, False): (6, 9), (1, False): (7, 10), (2, False): (8, 11),
    (3, True): (12,), (3, False): (13,),
}
_W_ONES = (14, 15)
_S_ROWS = {
    (0, True): (0, 6), (1, True): (1, 7), (2, True): (2, 8),
    (0, False): (3, 9), (1, False): (4, 10), (2, False): (5, 11),
    (3, True): (14,), (3, False): (15,),
}
_S_ONES = (12, 13)


def _build_point_set(nc, pre, psum, dst, dram, npts, identity, ones16,
                     is_query):
    """Fill dst [K, npts] fp16 from dram [npts, 3] fp32.

    Column m = cc*128 + p of dst holds point j = p*(npts//128) + cc.
    """
    c = npts // 128
    rows, ones_rows = (_W_ROWS, _W_ONES) if is_query else (_S_ROWS, _S_ONES)

    raw = pre.tile([128, c, 3], F32, tag="raw")
    nc.sync.dma_start(out=raw[:], in_=dram.rearrange("(p c) d -> p c d", c=c))
    sq = pre.tile([128, c, 3], F32, tag="sq")
    nc.vector.tensor_mul(sq[:], raw[:], raw[:])
    n32 = pre.tile([128, c], F32, tag="n32")
    nc.vector.tensor_add(n32[:], sq[:, :, 0], sq[:, :, 1])
    nc.vector.tensor_add(n32[:], n32[:], sq[:, :, 2])

    for idx in range(4):
        src = raw[:, :, idx] if idx < 3 else n32[:, :]
        pt = psum.tile([128, 512], F32, tag="grp")
        nc.tensor.transpose(pt[0:c, 0:128], src, identity[:])
        hi = pre.tile([128, 128], F16, tag="hi")
        lo = pre.tile([128, 128], F16, tag="lo")
        nc.vector.tensor_copy(hi[0:c, :], pt[0:c, 0:128])
        nc.vector.tensor_sub(lo[0:c, :], pt[0:c, 0:128], hi[0:c, :])
        if is_query and idx < 3:
            # -2*qh / -2*ql (exact doubling of the fp16 halves)
            nc.vector.tensor_scalar_mul(hi[0:c, :], hi[0:c, :], -2.0)
            nc.vector.tensor_scalar_mul(lo[0:c, :], lo[0:c, :], -2.0)
        for r in rows[(idx, True)]:
            nc.sync.dma_start(out=dst[r:r + 1, :], in_=hi[0:c, :])
        for r in rows[(idx, False)]:
            nc.sync.dma_start(out=dst[r:r + 1, :], in_=lo[0:c, :])
    for r in ones_rows:
        nc.sync.dma_start(out=dst[r:r + 1, :], in_=ones16[:, 0:c])


def _build_program():
    if "nc" in _CACHE:
        return _CACHE["nc"]

    nc = bacc.Bacc(None)
    gt_d = nc.declare_dram_parameter("gt", [NG_PTS, 3], F32, isOutput=False)
    fine_d = nc.declare_dram_parameter("fine", [NF_H, 3], F32, isOutput=False)
    coarse_d = nc.declare_dram_parameter("coarse", [NC_H, 3], F32,
                                         isOutput=False)
    out_d = nc.declare_dram_parameter("out", [128, OUT_COLS], F32,
                                      isOutput=True)

    with tile.TileContext(nc) as tc:
        import contextlib
        with contextlib.ExitStack() as ctx:
            singles = ctx.enter_context(tc.tile_pool(name="singles", bufs=1))
            pre = ctx.enter_context(tc.tile_pool(name="pre", bufs=3))
            scr = ctx.enter_context(tc.tile_pool(name="scr", bufs=3))
            rpp = ctx.enter_context(tc.tile_pool(name="rpp", bufs=3))
            psum = ctx.enter_context(
                tc.tile_pool(name="psum", bufs=2, space="PSUM"))

            identity = singles.tile([128, 128], F32)
            masks.make_identity(nc, identity[:])
            identity16 = singles.tile([128, 128], F16)
            nc.vector.tensor_copy(identity16[:], identity[:])
            ones16 = singles.tile([128, 64], F16)
            nc.gpsimd.memset(ones16[:], 1.0)

            s_gt = singles.tile([48, NG_PTS], F16)
            w_fine = singles.tile([48, NF_H], F16)
            w_coarse = singles.tile([48, NC_H], F16)
            m_dt = F16 if MODE == "fast" else F32
            m_init = 60000.0 if MODE == "fast" else BIG
            m_fine = singles.tile([128, NG_PTS], m_dt)
            nc.vector.memset(m_fine[:], m_init)
            m_coarse = singles.tile([128, NG_PTS], m_dt)
            nc.gpsimd.memset(m_coarse[:], m_init)
            m32 = None
            if MODE == "fast":
                m32 = singles.tile([128, NG_PTS], F32, tag="m32")
            rm_fine = singles.tile([128, FCH], F32)
            rm_coarse = singles.tile([128, CCH], F32)
            gt_vs_fine = singles.tile([128, TBLK], F32)
            gt_vs_coarse = singles.tile([128, TBLK], F32)

            _build_point_set(nc, pre, psum, s_gt, gt_d, NG_PTS, identity,
                             ones16, is_query=False)
            _build_point_set(nc, pre, psum, w_fine, fine_d, NF_H, identity,
                             ones16, is_query=True)
            _build_point_set(nc, pre, psum, w_coarse, coarse_d, NC_H,
                             identity, ones16, is_query=True)
            # replicate the K rows at partitions 32:48 for 2-way PE
            # row-group packing (two concurrent matmuls per pair)
            for t in (s_gt, w_fine, w_coarse):
                nc.sync.dma_start(out=t[32:32 + K, :], in_=t[0:K, :])

            gctr = 0
            for w, nch, m_state, rm in (
                (w_coarse, CCH, m_coarse, rm_coarse),
                (w_fine, FCH, m_fine, rm_fine),
            ):
                for cc in range(nch):
                    lhsT0 = w[0:K, cc * 128:(cc + 1) * 128]
                    lhsT1 = w[32:32 + K, cc * 128:(cc + 1) * 128]
                    if MODE == "fast":
                        # ACT copies each PSUM group into a chunk-wide fp16
                        # scratch; DVE then runs one col-min update and one
                        # fold-min tree over the whole 8192-wide scratch at
                        # the 2x 16-bit rate (PSUM is freed by the copy).
                        sc = scr.tile([128, NG_PTS], F16, tag="sc")
                    else:
                        rp = rpp.tile([128, NGRP], F32, tag="rp")
                    for g in range(NGRP):
                        ps = psum.tile([128, GRP], F32, tag="grp")
                        for jp in range(GRP // 1024):
                            j0 = 2 * jp
                            col = g * GRP + j0 * 512
                            nc.tensor.matmul(
                                ps[:, j0 * 512:(j0 + 1) * 512],
                                lhsT0,
                                s_gt[0:K, col:col + 512],
                                start=True, stop=True,
                            )
                            nc.tensor.matmul(
                                ps[:, (j0 + 1) * 512:(j0 + 2) * 512],
                                lhsT1,
                                s_gt[32:32 + K, col + 512:col + 1024],
                                start=True, stop=True,
                            )
                        if MODE == "fast":
                            nc.scalar.copy(sc[:, g * GRP:(g + 1) * GRP],
                                           ps[:])
                        else:
                            msl = m_state[:, g * GRP:(g + 1) * GRP]
                            nc.vector.tensor_reduce(
                                out=rp[:, g:g + 1], in_=ps[:],
                                axis=mybir.AxisListType.X,
                                op=mybir.AluOpType.min)
                            nc.vector.tensor_tensor(
                                out=msl, in0=ps[:], in1=msl,
                                op=mybir.AluOpType.min)
                        gctr += 1
                    if MODE == "fast":
                        nc.vector.tensor_tensor(
                            out=m_state[:], in0=sc[:], in1=m_state[:],
                            op=mybir.AluOpType.min)
                        wdt = NG_PTS // 2
                        while wdt >= 512:
                            nc.vector.tensor_tensor(
                                out=sc[:, 0:wdt], in0=sc[:, 0:wdt],
                                in1=sc[:, wdt:2 * wdt],
                                op=mybir.AluOpType.min)
                            wdt //= 2
                        nc.vector.tensor_reduce(
                            out=rm[:, cc:cc + 1], in_=sc[:, 0:2 * wdt],
                            axis=mybir.AxisListType.X,
                            op=mybir.AluOpType.min)
                    else:
                        nc.vector.tensor_reduce(
                            out=rm[:, cc:cc + 1], in_=rp[:],
                            axis=mybir.AxisListType.X,
                            op=mybir.AluOpType.min)

            # col-min extraction: transpose M blocks, reduce over original
            # partitions (=query chunk lanes) to get per-gt-point mins
            for m_state, gt_min in ((m_coarse, gt_vs_coarse),
                                    (m_fine, gt_vs_fine)):
                if MODE == "fast":
                    # fp16 transpose crashes the device; convert to fp32
                    # (on gpsimd -- it is idle and DVE is the bottleneck)
                    nc.gpsimd.tensor_copy(m32[:], m_state[:])
                    m_state = m32
                for t4 in range(TBLK // 4):
                    pt = psum.tile([128, 512], F32, tag="grp")
                    for j in range(4):
                        t = t4 * 4 + j
                        nc.tensor.transpose(
                            pt[:, j * 128:(j + 1) * 128],
                            m_state[:, t * 128:(t + 1) * 128],
                            identity[:])
                    nc.vector.tensor_reduce(
                        out=gt_min[:, t4 * 4:(t4 + 1) * 4],
                        in_=pt.rearrange("p (b f) -> p b f", f=128),
                        axis=mybir.AxisListType.X, op=mybir.AluOpType.min)

            c0 = 0
            for t in (rm_fine, rm_coarse, gt_vs_fine, gt_vs_coarse):
                w = t.shape[-1]
                nc.sync.dma_start(out=out_d[:, c0:c0 + w], in_=t[:])
                c0 += w

    nc.finalize()
    _CACHE["nc"] = nc
    return nc


def kernel(coarse, fine, gt, alpha):
    global LAST_EXEC_NS, LAST_RESULTS
    coarse = np.asarray(coarse, dtype=np.float32)
    fine = np.asarray(fine, dtype=np.float32)
    gt = np.asarray(gt, dtype=np.float32)

    nc = _build_program()

    in_maps = []
    for core in range(NCORES):
        b, h = divmod(core, 2)
        in_maps.append({
            "gt": np.ascontiguousarray(gt[b]),
            "fine": np.ascontiguousarray(fine[b, h * NF_H:(h + 1) * NF_H]),
            "coarse": np.ascontiguousarray(coarse[b, h * NC_H:(h + 1) * NC_H]),
        })

    trace = os.environ.get("CHAMFER_TRACE", "0") == "1"
    res = run_bass_kernel_spmd(nc, in_maps, list(range(NCORES)), trace=trace)
    LAST_EXEC_NS = res.exec_time_ns
    LAST_RESULTS = res

    mins_c = np.empty((B, NC_PTS), np.float32)
    mins_f = np.empty((B, NF_PTS), np.float32)
    gmin_f = np.empty((B, NG_PTS), np.float32)
    gmin_c = np.empty((B, NG_PTS), np.float32)
    for core in range(NCORES):
        b, h = divmod(core, 2)
        o = res.results[core]["out"]
        i0 = 0
        # rm[p, cc] = min for query point p*nch + cc  -> reshape is j-ordered
        rmf = o[:, i0:i0 + FCH].reshape(-1); i0 += FCH
        rmc = o[:, i0:i0 + CCH].reshape(-1); i0 += CCH
        # gt_min[p, t] = min for gt point p*64 + t -> reshape is j-ordered
        gf = o[:, i0:i0 + TBLK].reshape(-1); i0 += TBLK
        gc = o[:, i0:i0 + TBLK].reshape(-1)
        mins_f[b, h * NF_H:(h + 1) * NF_H] = rmf
        mins_c[b, h * NC_H:(h + 1) * NC_H] = rmc
        if h == 0:
            gmin_f[b] = gf
            gmin_c[b] = gc
        else:
            gmin_f[b] = np.minimum(gmin_f[b], gf)
            gmin_c[b] = np.minimum(gmin_c[b], gc)

    def srt(x):
        return np.sqrt(np.maximum(x, 0.0))

    loss_c = srt(gmin_c).mean(dtype=np.float64) \
        + 0.1 * srt(mins_c).mean(dtype=np.float64)
    loss_f = srt(gmin_f).mean(dtype=np.float64) \
        + 0.1 * srt(mins_f).mean(dtype=np.float64)
    return np.float32(loss_c + float(np.asarray(alpha)) * loss_f)



# revision 2
# speedup vs baseline: 1.1084x; 1.1084x over previous
"""Chamfer loss kernel for Trainium2 (8 NeuronCores, SPMD).

Problem: loss = cd(coarse, gt) + alpha * cd(fine, gt) where
  cd(x, gt) = mean(sqrt(min_x |gt - x|^2)) + 0.1 * mean(sqrt(min_gt |x - gt|^2))

Sharding: core i -> (batch b = i//2, half h = i%2). Each core processes its
half of the query rows (fine: 4096, coarse: 512) against the FULL gt set
(8192) of its batch, block-wise.

All distances are computed NEGATED (PSUM = -D = 2 q.g - |q|^2 - |g|^2) so
every reduction is a MAX (the DVE/ACT op set has no fused min variants that
survive on this silicon).  K=13 fp16 split-precision matmul: fp32 values are
split v = vh + vl (22 mantissa bits); the ql*gl cross term (~2^-22 relative)
is dropped:

  k 0-2 : W=2qh   S=gh      k  9: W=-nq_h  S=1
  k 3-5 : W=2qh   S=gl      k 10: W=-nq_l  S=1
  k 6-8 : W=2ql   S=gh      k 11: W=1      S=-ng_h
                            k 12: W=1      S=-ng_l

Per 128-query chunk: PE streams 4 PSUM groups of 2048 (2-way row-group
packing: rows 32:45 replicate 0:13, two concurrent matmuls); ACT drains
each group to an fp16 scratch (its only job - 0.96 ns/elem); DVE does the
column-state update (running max, 2x fp16 rate) and the row-max fold tree.
Fine chunks run first so the fine column-state extraction (ACT fp32 cast +
PE transpose + DVE reduce) overlaps the coarse chunks; only the small
coarse extraction is a tail.

Point order is permuted on-chip (contiguous DMA + PE transpose instead of
8192-descriptor strided DMAs); min is order-invariant so the host just
reshapes accordingly.  Host combines core halves, negates, clamps, sqrts,
means.
"""

import os
import sys

import numpy as np

for _p in ("/opt/trn_rl_repo",):
    if _p not in sys.path:
        sys.path.insert(0, _p)

import concourse.bacc as bacc
import concourse.tile as tile
from concourse import masks, mybir
from concourse.bass_utils import run_bass_kernel_spmd

F32 = mybir.dt.float32
F16 = mybir.dt.float16
Alu = mybir.AluOpType
Act = mybir.ActivationFunctionType
Axis = mybir.AxisListType

NEG_BIG = -60000.0


def _install_ntff_hook():
    """The agent image's antenv lacks axon_hooks, which disables NTFF
    profiling under axon. Recreate the module and wire the ctypes hook
    from the boot package so trace=True yields exec_time_ns."""
    try:
        from antenv.axon_hooks import get_axon_ntff_profile_hook  # noqa: F401
        return
    except ImportError:
        pass
    import types

    import antenv

    mod = types.ModuleType("antenv.axon_hooks")
    _holder = {}
    mod.set_axon_ntff_profile_hook = lambda h: _holder.__setitem__("h", h)
    mod.get_axon_ntff_profile_hook = lambda: _holder.get("h")
    sys.modules["antenv.axon_hooks"] = mod
    antenv.axon_hooks = mod
    try:
        if "/root/.axon_site" not in sys.path:
            sys.path.insert(0, "/root/.axon_site")
        from trn_agent_boot.trn_boot import _ntff_profile_via_ctypes
        hook = _ntff_profile_via_ctypes("/opt/axon/libaxon_pjrt.so")
        if hook is not None:
            mod.set_axon_ntff_profile_hook(hook)
    except Exception as e:  # profiling is best-effort; run still works
        print(f"ntff hook install failed: {e}", file=sys.stderr)


_install_ntff_hook()

# Problem constants (hardcoded per contract)
B = 4
NC_PTS = 1024  # coarse points per batch
NF_PTS = 8192  # fine points per batch
NG_PTS = 8192  # gt points per batch
NCORES = 8

NF_H = NF_PTS // 2  # 4096
NC_H = NC_PTS // 2  # 512

K = 13              # contraction rows of the split-precision matmul
GRP = 2048          # free-dim columns per PSUM group (4 banks)
NGRP = NG_PTS // GRP
FCH = NF_H // 128   # 32 fine chunks
CCH = NC_H // 128   # 4 coarse chunks
TBLK = NG_PTS // 128  # 64 transpose blocks for col-state extraction

OUT_COLS = FCH + CCH + TBLK + TBLK

LAST_EXEC_NS = None
LAST_RESULTS = None

_CACHE = {}

# (source_idx, is_hi) -> destination rows, for query (W) and gt (S) tiles.
# source_idx: 0..2 = x/y/z coordinate, 3 = squared norm.
_W_ROWS = {
    (0, True): (0, 3), (1, True): (1, 4), (2, True): (2, 5),
    (0, False): (6,), (1, False): (7,), (2, False): (8,),
    (3, True): (9,), (3, False): (10,),
}
_W_ONES = (11, 12)
_S_ROWS = {
    (0, True): (0, 6), (1, True): (1, 7), (2, True): (2, 8),
    (0, False): (3,), (1, False): (4,), (2, False): (5,),
    (3, True): (11,), (3, False): (12,),
}
_S_ONES = (9, 10)


def _build_point_set(nc, pre, psum, dst, dram, npts, identity, ones16,
                     is_query):
    """Fill dst [K, npts] fp16 from dram [npts, 3] fp32.

    Column m = cc*128 + p of dst holds point j = p*(npts//128) + cc.
    W rows: 2*qh/2*ql for coords, -nq_h/-nq_l for the norm, 1s.
    S rows: gh/gl coords, -ng_h/-ng_l norm, 1s.
    """
    c = npts // 128
    rows, ones_rows = (_W_ROWS, _W_ONES) if is_query else (_S_ROWS, _S_ONES)

    raw = pre.tile([128, c, 3], F32, tag="raw")
    nc.sync.dma_start(out=raw[:], in_=dram.rearrange("(p c) d -> p c d", c=c))
    sq = pre.tile([128, c, 3], F32, tag="sq")
    nc.vector.tensor_mul(sq[:], raw[:], raw[:])
    n32 = pre.tile([128, c], F32, tag="n32")
    nc.vector.tensor_add(n32[:], sq[:, :, 0], sq[:, :, 1])
    nc.vector.tensor_add(n32[:], n32[:], sq[:, :, 2])

    for idx in range(4):
        src = raw[:, :, idx] if idx < 3 else n32[:, :]
        pt = psum.tile([128, 512], F32, tag="grp")
        nc.tensor.transpose(pt[0:c, 0:128], src, identity[:])
        hi = pre.tile([128, 128], F16, tag="hi")
        lo = pre.tile([128, 128], F16, tag="lo")
        nc.vector.tensor_copy(hi[0:c, :], pt[0:c, 0:128])
        nc.vector.tensor_sub(lo[0:c, :], pt[0:c, 0:128], hi[0:c, :])
        if idx < 3:
            if is_query:
                # 2*qh / 2*ql (exact doubling of the fp16 halves)
                nc.vector.tensor_scalar_mul(hi[0:c, :], hi[0:c, :], 2.0)
                nc.vector.tensor_scalar_mul(lo[0:c, :], lo[0:c, :], 2.0)
        else:
            # negated norm rows on both sides
            nc.vector.tensor_scalar_mul(hi[0:c, :], hi[0:c, :], -1.0)
            nc.vector.tensor_scalar_mul(lo[0:c, :], lo[0:c, :], -1.0)
        for r in rows[(idx, True)]:
            nc.sync.dma_start(out=dst[r:r + 1, :], in_=hi[0:c, :])
        for r in rows[(idx, False)]:
            nc.sync.dma_start(out=dst[r:r + 1, :], in_=lo[0:c, :])
    for r in ones_rows:
        nc.sync.dma_start(out=dst[r:r + 1, :], in_=ones16[:, 0:c])


def _extract_colstate(nc, psum, m_state, mcast, gt_min, identity):
    """gt-side maxes: cast m_state to fp32 (ACT), PE-transpose 64 blocks,
    DVE max-reduce over the original query lanes."""
    nc.scalar.copy(out=mcast[:], in_=m_state[:])
    for t4 in range(TBLK // 4):
        pt = psum.tile([128, 512], F32, tag="grp")
        for j in range(4):
            t = t4 * 4 + j
            nc.tensor.transpose(
                pt[:, j * 128:(j + 1) * 128],
                mcast[:, t * 128:(t + 1) * 128],
                identity[:])
        nc.vector.tensor_reduce(
            out=gt_min[:, t4 * 4:(t4 + 1) * 4],
            in_=pt.rearrange("p (b f) -> p b f", f=128),
            axis=Axis.X, op=Alu.max)


def _build_program():
    if "nc" in _CACHE:
        return _CACHE["nc"]

    nc = bacc.Bacc(None)
    gt_d = nc.declare_dram_parameter("gt", [NG_PTS, 3], F32, isOutput=False)
    fine_d = nc.declare_dram_parameter("fine", [NF_H, 3], F32, isOutput=False)
    coarse_d = nc.declare_dram_parameter("coarse", [NC_H, 3], F32,
                                         isOutput=False)
    out_d = nc.declare_dram_parameter("out", [128, OUT_COLS], F32,
                                      isOutput=True)

    with tile.TileContext(nc) as tc:
        import contextlib
        with contextlib.ExitStack() as ctx:
            singles = ctx.enter_context(tc.tile_pool(name="singles", bufs=1))
            pre = ctx.enter_context(tc.tile_pool(name="pre", bufs=3))
            scr = ctx.enter_context(tc.tile_pool(name="scr", bufs=3))
            psum = ctx.enter_context(
                tc.tile_pool(name="psum", bufs=2, space="PSUM"))

            identity = singles.tile([128, 128], F32)
            masks.make_identity(nc, identity[:])
            ones16 = singles.tile([128, 64], F16)
            nc.gpsimd.memset(ones16[:], 1.0)

            s_gt = singles.tile([48, NG_PTS], F16)
            w_fine = singles.tile([48, NF_H], F16)
            w_coarse = singles.tile([48, NC_H], F16)
            m_fine = singles.tile([128, NG_PTS], F16)
            nc.vector.memset(m_fine[:], NEG_BIG)
            m_coarse = singles.tile([128, NG_PTS], F16)
            nc.gpsimd.memset(m_coarse[:], NEG_BIG)
            mcast = singles.tile([128, NG_PTS], F32)
            rm_fine = singles.tile([128, FCH], F32)
            rm_coarse = singles.tile([128, CCH], F32)
            gt_vs_fine = singles.tile([128, TBLK], F32)
            gt_vs_coarse = singles.tile([128, TBLK], F32)

            _build_point_set(nc, pre, psum, s_gt, gt_d, NG_PTS, identity,
                             ones16, is_query=False)
            _build_point_set(nc, pre, psum, w_fine, fine_d, NF_H, identity,
                             ones16, is_query=True)
            _build_point_set(nc, pre, psum, w_coarse, coarse_d, NC_H,
                             identity, ones16, is_query=True)
            # replicate the K rows at partitions 32:32+K for 2-way PE
            # row-group packing (two concurrent matmuls per pair)
            for t in (s_gt, w_fine, w_coarse):
                nc.sync.dma_start(out=t[32:32 + K, :], in_=t[0:K, :])

            def do_chunk(w, cc, m_state, rm):
                lhsT0 = w[0:K, cc * 128:(cc + 1) * 128]
                lhsT1 = w[32:32 + K, cc * 128:(cc + 1) * 128]
                sc = scr.tile([128, NG_PTS], F16, tag="sc")
                for g in range(NGRP):
                    ps = psum.tile([128, GRP], F32, tag="grp")
                    for jp in range(GRP // 1024):
                        j0 = 2 * jp
                        col = g * GRP + j0 * 512
                        nc.tensor.matmul(
                            ps[:, j0 * 512:(j0 + 1) * 512],
                            lhsT0,
                            s_gt[0:K, col:col + 512],
                            start=True, stop=True,
                        )
                        nc.tensor.matmul(
                            ps[:, (j0 + 1) * 512:(j0 + 2) * 512],
                            lhsT1,
                            s_gt[32:32 + K, col + 512:col + 1024],
                            start=True, stop=True,
                        )
                    nc.scalar.copy(out=sc[:, g * GRP:(g + 1) * GRP],
                                   in_=ps[:])
                # column-state running max (2x fp16)
                nc.vector.tensor_tensor(
                    out=m_state[:], in0=sc[:], in1=m_state[:], op=Alu.max)
                # row-max fold tree (2x fp16), then a small 1x reduce
                wdt = NG_PTS // 2
                while wdt >= 512:
                    nc.vector.tensor_tensor(
                        out=sc[:, 0:wdt], in0=sc[:, 0:wdt],
                        in1=sc[:, wdt:2 * wdt], op=Alu.max)
                    wdt //= 2
                nc.vector.tensor_reduce(
                    out=rm[:, cc:cc + 1], in_=sc[:, 0:2 * wdt],
                    axis=Axis.X, op=Alu.max)

            # fine chunks first: the fine column-state extraction then
            # overlaps the coarse chunks, leaving only a tiny tail.
            for cc in range(FCH):
                do_chunk(w_fine, cc, m_fine, rm_fine)
            _extract_colstate(nc, psum, m_fine, mcast, gt_vs_fine, identity)
            for cc in range(CCH):
                do_chunk(w_coarse, cc, m_coarse, rm_coarse)
            _extract_colstate(nc, psum, m_coarse, mcast, gt_vs_coarse,
                              identity)

            c0 = 0
            for t in (rm_fine, rm_coarse, gt_vs_fine, gt_vs_coarse):
                w = t.shape[-1]
                nc.sync.dma_start(out=out_d[:, c0:c0 + w], in_=t[:])
                c0 += w

    nc.finalize()
    _CACHE["nc"] = nc
    return nc


def kernel(coarse, fine, gt, alpha):
    global LAST_EXEC_NS, LAST_RESULTS
    coarse = np.asarray(coarse, dtype=np.float32)
    fine = np.asarray(fine, dtype=np.float32)
    gt = np.asarray(gt, dtype=np.float32)

    nc = _build_program()

    in_maps = []
    for core in range(NCORES):
        b, h = divmod(core, 2)
        in_maps.append({
            "gt": np.ascontiguousarray(gt[b]),
            "fine": np.ascontiguousarray(fine[b, h * NF_H:(h + 1) * NF_H]),
            "coarse": np.ascontiguousarray(coarse[b, h * NC_H:(h + 1) * NC_H]),
        })

    trace = os.environ.get("CHAMFER_TRACE", "0") == "1"
    res = run_bass_kernel_spmd(nc, in_maps, list(range(NCORES)), trace=trace)
    LAST_EXEC_NS = res.exec_time_ns
    LAST_RESULTS = res

    mins_c = np.empty((B, NC_PTS), np.float32)
    mins_f = np.empty((B, NF_PTS), np.float32)
    gmin_f = np.empty((B, NG_PTS), np.float32)
    gmin_c = np.empty((B, NG_PTS), np.float32)
    for core in range(NCORES):
        b, h = divmod(core, 2)
        o = res.results[core]["out"]
        i0 = 0
        # rm[p, cc] = -min for query point p*nch + cc -> reshape is j-ordered
        rmf = o[:, i0:i0 + FCH].reshape(-1); i0 += FCH
        rmc = o[:, i0:i0 + CCH].reshape(-1); i0 += CCH
        # gt_min[p, t] = -min for gt point p*64 + t -> reshape is j-ordered
        gf = o[:, i0:i0 + TBLK].reshape(-1); i0 += TBLK
        gc = o[:, i0:i0 + TBLK].reshape(-1)
        mins_f[b, h * NF_H:(h + 1) * NF_H] = -rmf
        mins_c[b, h * NC_H:(h + 1) * NC_H] = -rmc
        if h == 0:
            gmin_f[b] = -gf
            gmin_c[b] = -gc
        else:
            gmin_f[b] = np.minimum(gmin_f[b], -gf)
            gmin_c[b] = np.minimum(gmin_c[b], -gc)

    def srt(x):
        return np.sqrt(np.maximum(x, 0.0))

    loss_c = srt(gmin_c).mean(dtype=np.float64) \
        + 0.1 * srt(mins_c).mean(dtype=np.float64)
    loss_f = srt(gmin_f).mean(dtype=np.float64) \
        + 0.1 * srt(mins_f).mean(dtype=np.float64)
    return np.float32(loss_c + float(np.asarray(alpha)) * loss_f)
